# revision 6
# baseline (speedup 1.0000x reference)
"""Trainium2 Bass kernel for a continuous bilinear Koopman operator rollout.

Problem (hardcoded shapes): z0 [256, 256] f32, kernel [256, 256] f32,
log_dt scalar, T=512.  Output: [256, 512, 256] f32 with
out[:, t, :] = z0 @ K_discrete^(t+1),
K_discrete = (I - 0.5*dt*K)^-1 (I + 0.5*dt*K), dt = exp(log_dt).

Strategy:
  - Host computes K_discrete (small [d,d] solve, as the sharding hint
    suggests) and a handful of its powers: A^1..A^16 and A^(16*2^i).
  - z0 and the [B, T, D] output are sharded across 8 cores along batch
    (32 trajectories per core) -- pure data parallelism.
  - On device, the T=512 serial recurrence is restructured as:
      phase B: chunk-start states sT_k = (z0 @ A^(16k)).T for k=0..31,
               built in 5 doubling rounds (s_{k+m} = s_k @ A^(16m)).
      phase C: out rows for chunk k, step j:  s_k @ A^j, j=1..16,
               as matmuls with M=128 (4 chunks x 32 batch) and N=512
               (two consecutive powers) -> PSUM -> SBUF -> 16KB-contiguous
               DMA into out[b, t, :].
  - float32r matmuls: fp32 bits in memory, single-pass PE (1 cycle/row at
    N>=256) instead of float32's LOW/HIGH double pass (4 cycles/row).
"""

import numpy as np

B = 256
D = 256
T = 512
N_CORES = 8
B_LOC = B // N_CORES      # 32
C = 16                    # chunk length (powers A^1..A^C shipped)
N_CHUNKS = T // C         # 32
N_GROUPS = N_CHUNKS // 4  # 8 groups of 4 chunks -> M=128
JP = C // 2               # 8 pairs of consecutive powers -> N=512

_CACHE = {}


def _build_bass():
    import concourse.tile as tile
    from concourse import bacc, mybir

    f32 = mybir.dt.float32
    f32r = mybir.dt.float32r
    nc = bacc.Bacc("TRN2", target_bir_lowering=False, debug=False)

    z0t = nc.dram_tensor("z0t", [D, B_LOC], f32r, kind="ExternalInput").ap()
    # prhs[h, :, (j-1)*256 : j*256] = A^j[h*128:(h+1)*128, :]   j=1..16
    prhs = nc.dram_tensor("prhs", [2, 128, C * D], f32r, kind="ExternalInput").ap()
    # qpow[:, i*512 + h*256 + c] = A^(16*2^i)[h*128 + r, c]     i=0..4
    qpow = nc.dram_tensor("qpow", [128, 5 * 2 * D], f32r, kind="ExternalInput").ap()
    out = nc.dram_tensor("out", [B_LOC, T, D], f32, kind="ExternalOutput").ap()
    # out_r[k, b, j*256 + d] = out[b, 16k + j, d]
    out_r = out.rearrange("b (k j) d -> k b (j d)", j=C)

    with tile.TileContext(nc) as tc:
        with (
            tc.tile_pool(name="const", bufs=1) as cpool,
            tc.tile_pool(name="psum", bufs=8, space="PSUM") as psum_pool,
            tc.tile_pool(name="stage", bufs=3) as stage_pool,
        ):
            # Persistent SBUF tiles.
            # S[h][:, k*32 + b] = s_k[b, h*128 + d']  (chunk starts, transposed)
            S = [
                cpool.tile([128, N_CHUNKS * B_LOC], f32r, name=f"s{h}")
                for h in range(2)
            ]
            P = [cpool.tile([128, C * D], f32r, name=f"p{h}") for h in range(2)]
            Q = cpool.tile([128, 5 * 2 * D], f32r, name="q")

            # Small loads (phase B prerequisites) on the SP HWDGE ring.
            for h in range(2):
                nc.sync.dma_start(S[h][:, 0:B_LOC], z0t[h * 128:(h + 1) * 128, :])
            nc.sync.dma_start(Q[:], qpow[:])
            # Big P loads on the ACT HWDGE ring, halved so phase C's first
            # half-group doesn't wait for the whole 4MB. Order: the lower
            # jp halves of both h first.
            for lohi in range(2):
                for h in range(2):
                    sl = slice(lohi * 2048, (lohi + 1) * 2048)
                    nc.scalar.dma_start(P[h][:, sl], prhs[h, :, sl])

            # Phase B: doubling rounds. Round i: for k in [0, m),
            #   sT_{k+m} = (A^(16m)).T @ sT_k,  m = 2^i.
            for i in range(5):
                m = 1 << i
                n = B_LOC * m
                for ho in range(2):
                    ps = psum_pool.tile([128, 512], f32, name="psb", tag="ps")
                    for h in range(2):
                        nc.tensor.matmul(
                            ps[:, 0:n],
                            Q[:, i * 512 + h * D + ho * 128:
                               i * 512 + h * D + (ho + 1) * 128],
                            S[h][:, 0:n],
                            start=(h == 0),
                            stop=(h == 1),
                        )
                    nc.vector.tensor_copy(S[ho][:, n:2 * n], ps[:, 0:n])

            # Phase C: group g covers chunks 4g..4g+3 (M = 4 chunks x 32
            # batch = 128 rows).  Split into two halves of 4 jp's each so
            # the stationary operand S[h]-block is reloaded only once per
            # 4 matmuls, and PSUM pressure stays at 4 banks per half.
            for g in range(N_GROUPS):
                stage = stage_pool.tile([128, C * D], f32, name="stage")
                for half in range(2):
                    jps = [half * 4 + q for q in range(4)]
                    pss = {
                        jp: psum_pool.tile([128, 512], f32, name="psc", tag="ps")
                        for jp in jps
                    }
                    for h in range(2):
                        for jp in jps:
                            nc.tensor.matmul(
                                pss[jp][:],
                                S[h][:, g * 128:(g + 1) * 128],
                                P[h][:, jp * 512:(jp + 1) * 512],
                                start=(h == 0),
                                stop=(h == 1),
                                skip_group_check=True,
                            )
                    for idx, jp in enumerate(jps):
                        dst = stage[:, jp * 512:(jp + 1) * 512]
                        if idx == 1:
                            nc.scalar.copy(dst, pss[jp][:])
                        else:
                            nc.vector.tensor_copy(dst, pss[jp][:])
                    # Drain this half: 1MB, 128 partitions, 8KB contiguous
                    # per partition row.  Alternate the two HWDGE rings.
                    dma_eng = nc.sync if (g + half) % 2 == 0 else nc.scalar
                    dma_eng.dma_start(
                        out_r[4 * g:4 * (g + 1), :, half * 2048:(half + 1) * 2048],
                        stage[:, half * 2048:(half + 1) * 2048],
                    )

    nc.compile()
    return nc


def _host_prep(z0, kernel, log_dt):
    """fp64 host math: K_discrete and its needed powers."""
    K = np.asarray(kernel, dtype=np.float64)
    dt = float(np.exp(np.float64(np.asarray(log_dt))))
    eye = np.eye(D, dtype=np.float64)
    A = np.linalg.solve(eye - 0.5 * dt * K, eye + 0.5 * dt * K)

    pows = [None] * (C + 1)  # pows[j] = A^j
    pows[1] = A
    for j in range(2, C + 1):
        pows[j] = pows[j - 1] @ A

    # qs[i] = A^(C * 2^i), i = 0..4
    qs = [pows[C]]
    for _ in range(4):
        qs.append(qs[-1] @ qs[-1])

    # prhs [2, 128, C*D]
    parr = np.stack([pows[j] for j in range(1, C + 1)], axis=0)  # [16, 256, 256]
    prhs = np.ascontiguousarray(
        parr.reshape(C, 2, 128, D).transpose(1, 2, 0, 3).reshape(2, 128, C * D)
    ).astype(np.float32)

    # qpow [128, 5*2*D]: qpow[r, i*512 + h*256 + c] = qs[i][h*128 + r, c]
    qarr = np.stack(qs, axis=0)  # [5, 256, 256]
    qpow = np.ascontiguousarray(
        qarr.reshape(5, 2, 128, D).transpose(2, 0, 1, 3).reshape(128, 5 * 2 * D)
    ).astype(np.float32)

    z0 = np.asarray(z0, dtype=np.float32)
    z0t_shards = [
        np.ascontiguousarray(z0[c * B_LOC:(c + 1) * B_LOC, :].T) for c in range(N_CORES)
    ]
    return z0t_shards, prhs, qpow


def kernel(**inputs):
    from concourse.bass_utils import run_bass_kernel_spmd

    z0 = inputs["z0"]
    kmat = inputs["kernel"]
    log_dt = inputs["log_dt"]
    t_in = int(np.asarray(inputs["T"]))
    assert t_in == T, f"kernel hardcoded for T={T}, got {t_in}"
    assert tuple(np.shape(z0)) == (B, D)

    z0t_shards, prhs, qpow = _host_prep(z0, kmat, log_dt)

    if "nc" not in _CACHE:
        _CACHE["nc"] = _build_bass()
    nc = _CACHE["nc"]

    in_maps = [
        {"z0t": z0t_shards[c], "prhs": prhs, "qpow": qpow} for c in range(N_CORES)
    ]
    res = run_bass_kernel_spmd(nc, in_maps, core_ids=list(range(N_CORES)))
    return np.concatenate([res.results[c]["out"] for c in range(N_CORES)], axis=0)
